# revision 7
# baseline (speedup 1.0000x reference)
"""LFADS GenGRU cell + source attention + factor readout, 8-way data-parallel
over batch on Trainium2 (Bass/Tile).

Layout: every activation lives in SBUF as [features -> partitions,
batch -> free] ("T-layout"). All weights are host-pre-transposed to
[in_features, out_features] so each matmul is
    outT[o_tile, b] = W_T[:, o_tile].T @ inT[:, b]
with the contraction on partitions and no on-chip transposes anywhere.

Attention softmax (over S=64 source steps) runs in [e -> partitions,
(b, s) -> free] layout and is engine-balanced:
  - gen_alpha[e, b] is accumulated straight into the score PSUM via a
    rank-128 matmul against an identity whose rhs AP broadcasts over s
    (lhsT = gen_alpha in natural [b, e] layout), so no DVE broadcast-add.
  - exp runs on ScalarE directly from a 4-bank PSUM group; relu is replaced
    by the exact identity exp(relu(x)) = 1 + relu(exp(x) - 1): the +1
    offsets become a constant 64 in the softmax denominator and a
    host-precomputed sum_s src term in the numerator. relu(ex-1)
    alternates between ScalarE and VectorE to balance the two engines.
  - ctx and the denominator come from one interleaved [b, {ex2, ex2*src}, s]
    tile, reduced over s with a short bf16 add-tree + one f32-output reduce.

Matmul operands are bf16 (fp32 PSUM accumulation); fp32 is kept on the
gen/u/c elementwise path so the `gen` output only sees matmul rounding.
"""

import os
import sys

sys.path.insert(0, "/opt/trn_rl_repo")

import numpy as np
import ml_dtypes

import concourse.bacc as bacc
import concourse.mybir as mybir
import concourse.tile as tile
from concourse import bass_utils

B, S, IN, G, SRC, F = 1024, 64, 256, 1024, 256, 128
NCORES = 8
BL = B // NCORES  # 128, batch per core == partition count
P = 128
CB = 8  # batch columns per attention chunk (CB * S = 512 = one PSUM bank)
NCH = BL // CB  # 16 chunks per e-tile
QC = 4  # chunks per quarter = one 4-bank PSUM group

F32 = mybir.dt.float32
BF16 = mybir.dt.bfloat16
AF = mybir.ActivationFunctionType
OP = mybir.AluOpType
X = mybir.AxisListType.X

_CACHE: dict = {}
LAST_EXEC_NS = None


def _mm(nc, ps, lhsT, rhs, first, last):
    nc.tensor.matmul(ps, lhsT, rhs, start=first, stop=last)


def _build():
    nc = bacc.Bacc("TRN2", target_bir_lowering=False, debug=False)

    def din(name, shape, dt=BF16):
        return nc.dram_tensor(name, shape, dt, kind="ExternalInput").ap()

    # blobA: xT | hTbf | WxruT | WhruT   (cols: 256 | 1024 | 4096 | 16384)
    NA = 2 * BL + 8 * BL + 2 * 2 * G + 8 * 2 * G
    # blobB: WxcT | WrhcT | WagT | WasT | ident  (2048 | 8192 | 2048 | 512 | 128)
    NB = 2 * G + 8 * G + 8 * SRC + 2 * SRC + P
    # blobD: WogT | WosT | WfacT  (8192 | 2048 | 1024)
    ND = 8 * G + 2 * G + 8 * F
    blobA_d = din("blobA", (P, NA))
    blobB_d = din("blobB", (P, NB))
    blobD_d = din("blobD", (P, ND))
    hT_d = din("hT", (G, BL), F32)
    src_d = din("srcT2", (SRC, BL, S))
    bias_d = din("biases", (P, 25), F32)
    ssum_d = din("srcSumT", (SRC, BL), F32)
    gen_d = nc.dram_tensor("genT_out", (G, BL), F32, kind="ExternalOutput").ap()
    fac_d = nc.dram_tensor("facT_out", (F, BL), F32, kind="ExternalOutput").ap()

    kp = lambda ap: ap.rearrange("(k p) n -> p k n", p=P)  # [K*P, N] -> [P, K, N]

    with tile.TileContext(nc) as tc:
        with (
            tc.tile_pool(name="weights", bufs=1) as wp,
            tc.tile_pool(name="acts", bufs=1) as app,
            tc.tile_pool(name="tmp", bufs=4) as tp,
            tc.tile_pool(name="combo_p", bufs=2) as cbp,
            tc.tile_pool(name="psum", bufs=2, space="PSUM") as psp,
        ):
            # ---- persistent loads: few big blob DMAs (SP issue is ~2.2us each) ----
            blobA = wp.tile([P, NA], BF16, name="blobA")
            nc.sync.dma_start(blobA[:], blobA_d[:])
            bias_sb = wp.tile([P, 25], F32, name="bias_sb")
            nc.sync.dma_start(bias_sb[:], bias_d[:])
            hT_sb = wp.tile([P, 8, BL], F32, name="hT_sb")
            nc.sync.dma_start(hT_sb[:], kp(hT_d))
            blobB = wp.tile([P, NB], BF16, name="blobB")
            nc.sync.dma_start(blobB[:], blobB_d[:])
            src_sb = wp.tile([P, 2, BL, S], BF16, name="src_sb")
            nc.sync.dma_start(src_sb[:], src_d.rearrange("(k p) b s -> p k b s", p=P))
            ssum_sb = wp.tile([P, 2, BL], F32, name="ssum_sb")
            nc.sync.dma_start(ssum_sb[:], kp(ssum_d))
            blobD = wp.tile([P, ND], BF16, name="blobD")
            nc.sync.dma_start(blobD[:], blobD_d[:])

            def seg(blob, off, shape):
                n = 1
                for d in shape:
                    n *= d
                ap = blob[:, off:off + n]
                if len(shape) > 1:
                    ap = ap.rearrange("p (a b) -> p a b", b=shape[-1]) if len(shape) == 2 else ap.rearrange(
                        "p (a b c) -> p a b c", b=shape[1], c=shape[2])
                return ap, off + n

            o = 0
            xT_sb, o = seg(blobA, o, (2, BL))
            h_bf, o = seg(blobA, o, (8, BL))
            wxru_sb, o = seg(blobA, o, (2, 2 * G))
            whru_sb, o = seg(blobA, o, (8, 2 * G))
            o = 0
            wxc_sb, o = seg(blobB, o, (2, G))
            wrhc_sb, o = seg(blobB, o, (8, G))
            wag_sb, o = seg(blobB, o, (8, SRC))
            was_sb, o = seg(blobB, o, (2, SRC))
            ident_sb = blobB[:, o:o + P]; o += P
            o = 0
            wog_sb, o = seg(blobD, o, (8, G))
            wos_sb, o = seg(blobD, o, (2, G))
            wfac_sb, o = seg(blobD, o, (8, F))

            rh_bf = app.tile([P, 8, BL], BF16, name="rh_bf")
            u_f = app.tile([P, 8, BL], F32, name="u_f")
            gen_f = app.tile([P, 8, BL], F32, name="gen_f")
            gen_bf = app.tile([P, 8, BL], BF16, name="gen_bf")

            # ---- ru = x @ WxruT + h @ WhruT + b; r = sig(ru_r), u = sig(ru_u + 1) ----
            for m in range(16):
                if m % 2 == 0:
                    rups = psp.tile([P, 16 * P], F32, tag="ps", name=f"rups{m}")
                sl = slice((m % 2) * 512, (m % 2) * 512 + P)
                wsl = slice(m * P, (m + 1) * P)
                for k in range(2):
                    _mm(nc, rups[:, sl], wxru_sb[:, k, wsl], xT_sb[:, k, :], k == 0, False)
                for k in range(8):
                    _mm(nc, rups[:, sl], whru_sb[:, k, wsl], h_bf[:, k, :], False, k == 7)
                if m < 8:
                    r_t = tp.tile([P, BL], F32, tag="r_t")
                    nc.scalar.activation(r_t[:], rups[:, sl], AF.Sigmoid, bias=bias_sb[:, m:m + 1])
                    nc.vector.tensor_tensor(rh_bf[:, m, :], r_t[:], hT_sb[:, m, :], op=OP.mult)
                else:
                    nc.scalar.activation(u_f[:, m - 8, :], rups[:, sl], AF.Sigmoid, bias=bias_sb[:, m:m + 1])

            # ---- c = tanh(x @ WxcT + rh @ WrhcT + b); gen = clip(c + u*(h-c)) ----
            for m in range(8):
                if m % 2 == 0:
                    cps = psp.tile([P, 16 * P], F32, tag="ps", name=f"cps{m}")
                sl = slice((m % 2) * 512, (m % 2) * 512 + P)
                wsl = slice(m * P, (m + 1) * P)
                for k in range(2):
                    _mm(nc, cps[:, sl], wxc_sb[:, k, wsl], xT_sb[:, k, :], k == 0, False)
                for k in range(8):
                    _mm(nc, cps[:, sl], wrhc_sb[:, k, wsl], rh_bf[:, k, :], False, k == 7)
                c_t = tp.tile([P, BL], F32, tag="c_t")
                nc.scalar.activation(c_t[:], cps[:, sl], AF.Tanh, bias=bias_sb[:, 16 + m:17 + m])
                d_t = tp.tile([P, BL], F32, tag="d_t")
                nc.vector.tensor_tensor(d_t[:], hT_sb[:, m, :], c_t[:], op=OP.subtract)
                nc.vector.tensor_tensor(d_t[:], u_f[:, m, :], d_t[:], op=OP.mult)
                nc.vector.tensor_tensor(gen_f[:, m, :], c_t[:], d_t[:], op=OP.add)
                nc.vector.tensor_scalar(gen_f[:, m, :], gen_f[:, m, :], 5.0, -5.0, op0=OP.min, op1=OP.max)
                nc.scalar.copy(gen_bf[:, m, :], gen_f[:, m, :])
            nc.sync.dma_start(kp(gen_d), gen_f[:])

            # ---- gen_alpha in natural [b, e] layout: lhsT = genT tiles ----
            gaps_t = psp.tile([P, 16 * P], F32, tag="ps", name="gaps")
            for k in range(8):
                _mm(nc, gaps_t[:, :SRC], gen_bf[:, k, :], wag_sb[:, k, :], k == 0, k == 7)
            ga_nat = app.tile([P, SRC], BF16, name="ga_nat")
            nc.scalar.copy(ga_nat[:], gaps_t[:, :SRC])

            # ---- attention: psum scores -> exp -> relu(ex-1) -> interleaved reduce ----
            # se_ct[et][:, b, 0] = sum_s relu(ex-1);  [..., 1] = sum_s relu(ex-1)*src
            se_ct = [app.tile([P, BL, 2], F32, name=f"se_ct{et}") for et in range(2)]
            NQ = NCH // QC
            for et in range(2):
                esl = slice(et * P, (et + 1) * P)
                for q in range(NQ):
                    chs = [q * QC + i for i in range(QC)]
                    qbs = slice(q * QC * CB, (q + 1) * QC * CB)  # 32 b columns
                    ps = psp.tile([P, QC * CB * S], F32, tag="ps", name=f"aps{et}_{q}")
                    for dk in range(2):
                        for i, ch in enumerate(chs):
                            rhs = src_sb[:, dk, ch * CB:(ch + 1) * CB, :].rearrange("p b s -> p (b s)")
                            _mm(nc, ps[:, i * CB * S:(i + 1) * CB * S], was_sb[:, dk, esl], rhs, dk == 0, False)
                    for i, ch in enumerate(chs):
                        rhs = ident_sb[:, ch * CB:(ch + 1) * CB].unsqueeze(2).broadcast_to([P, CB, S])
                        _mm(nc, ps[:, i * CB * S:(i + 1) * CB * S], ga_nat[:, esl], rhs, False, True)
                    ex = tp.tile([P, QC * CB, S], BF16, tag="ex")
                    nc.scalar.activation(ex[:], ps.rearrange("p (b s) -> p b s", s=S), AF.Exp)
                    combo = cbp.tile([P, QC * CB, 2, S], BF16, tag="combo")
                    if q % 2 == 0:
                        nc.vector.tensor_scalar(combo[:, :, 0, :], ex[:], 1.0, 0.0, op0=OP.subtract, op1=OP.max)
                    else:
                        nc.scalar.activation(combo[:, :, 0, :], ex[:], AF.Relu, bias=bias_sb[:, 24:25])
                    nc.vector.tensor_tensor(combo[:, :, 1, :], combo[:, :, 0, :], src_sb[:, et, qbs, :], op=OP.mult)
                    # reduce over s: short bf16 add-tree, then one f32-output reduce
                    nc.gpsimd.tensor_tensor(combo[:, :, :, 0:32], combo[:, :, :, 0:32], combo[:, :, :, 32:64], op=OP.add)
                    nc.vector.tensor_tensor(combo[:, :, :, 0:16], combo[:, :, :, 0:16], combo[:, :, :, 16:32], op=OP.add)
                    nc.vector.tensor_tensor(combo[:, :, :, 0:8], combo[:, :, :, 0:8], combo[:, :, :, 8:16], op=OP.add)
                    nc.vector.reduce_sum(se_ct[et][:, qbs, :], combo[:, :, :, 0:8], axis=X)

            # softmax epilogue:
            #   sumexp = 64 + sum relu(ex-1);  ctx = (src_sum + sum relu(ex-1)*src) / sumexp
            ctx_bf = app.tile([P, 2, BL], BF16, name="ctx_bf")
            for et in range(2):
                den = tp.tile([P, BL], F32, tag="den")
                nc.vector.tensor_scalar(den[:], se_ct[et][:, :, 0], float(S), None, op0=OP.add)
                rcp = tp.tile([P, BL], F32, tag="rcp")
                nc.vector.reciprocal(rcp[:], den[:])
                num = tp.tile([P, BL], F32, tag="num")
                nc.vector.tensor_tensor(num[:], ssum_sb[:, et, :], se_ct[et][:, :, 1], op=OP.add)
                nc.vector.tensor_tensor(ctx_bf[:, et, :], num[:], rcp[:], op=OP.mult)

            # ---- attn_out = tanh(WogT.T @ genT + WosT.T @ ctxT) ----
            attn_bf = app.tile([P, 8, BL], BF16, name="attn_bf")
            for m in range(8):
                if m % 2 == 0:
                    aops = psp.tile([P, 16 * P], F32, tag="ps", name=f"aops{m}")
                sl = slice((m % 2) * 512, (m % 2) * 512 + P)
                wsl = slice(m * P, (m + 1) * P)
                for k in range(8):
                    _mm(nc, aops[:, sl], wog_sb[:, k, wsl], gen_bf[:, k, :], k == 0, False)
                for k in range(2):
                    _mm(nc, aops[:, sl], wos_sb[:, k, wsl], ctx_bf[:, k, :], False, k == 1)
                nc.scalar.activation(attn_bf[:, m, :], aops[:, sl], AF.Tanh)

            # ---- factors = WfacT.T @ attn_outT ----
            fps = psp.tile([P, 16 * P], F32, tag="ps", name="fps")
            for k in range(8):
                _mm(nc, fps[:, :P], wfac_sb[:, k, :], attn_bf[:, k, :], k == 0, k == 7)
            fac_sb = app.tile([P, BL], F32, name="fac_sb")
            nc.scalar.copy(fac_sb[:], fps[:, :P])
            nc.sync.dma_start(fac_d[:], fac_sb[:])

    nc.compile()
    return nc


def _host_prep(inputs):
    bf = ml_dtypes.bfloat16

    def t(a):
        return np.ascontiguousarray(np.asarray(a, dtype=np.float32).T)

    x = np.asarray(inputs["x"], np.float32)
    h = np.asarray(inputs["h"], np.float32)
    src = np.asarray(inputs["src"], np.float32)
    W_alpha = np.asarray(inputs["W_alpha"], np.float32)
    W_out = np.asarray(inputs["W_out"], np.float32)
    b_hru = np.asarray(inputs["b_hru"], np.float32)
    b_rhc = np.asarray(inputs["b_rhc"], np.float32)

    xT = t(x).astype(bf)                       # [IN, B]
    hT = t(h)                                  # [G, B] fp32
    srcT2 = np.ascontiguousarray(src.transpose(2, 1, 0)).astype(bf)  # [SRC, B, S]
    # the kernel multiplies attention weights against bf16 src, and the
    # numerator offset must match that rounding exactly
    srcSumT = srcT2.astype(np.float32).sum(axis=2)  # [SRC, B] fp32

    def kp_np(wt, K):
        # [K*P, N] -> [P, K*N]
        a = np.asarray(wt)
        N = a.shape[1]
        return a.reshape(K, P, N).transpose(1, 0, 2).reshape(P, K * N)

    WxruT = t(inputs["W_xru"]).astype(bf)
    WhruT = t(inputs["W_hru"]).astype(bf)
    WxcT = t(inputs["W_xc"]).astype(bf)
    WrhcT = t(inputs["W_rhc"]).astype(bf)
    WagT = np.ascontiguousarray(W_alpha[:, :G].T).astype(bf)
    WasT = np.ascontiguousarray(W_alpha[:, G:].T).astype(bf)
    WogT = np.ascontiguousarray(W_out[:, :G].T).astype(bf)
    WosT = np.ascontiguousarray(W_out[:, G:].T).astype(bf)
    WfacT = t(inputs["W_fac"]).astype(bf)
    identb = np.eye(P, dtype=np.float32).astype(bf)

    blobB = np.concatenate(
        [kp_np(WxcT, 2), kp_np(WrhcT, 8), kp_np(WagT, 8), kp_np(WasT, 2), identb],
        axis=1)
    blobD = np.concatenate(
        [kp_np(WogT, 8), kp_np(WosT, 2), kp_np(WfacT, 8)], axis=1)
    blobA_w = np.concatenate([kp_np(WxruT, 2), kp_np(WhruT, 8)], axis=1)

    shared = {
        "blobB": np.ascontiguousarray(blobB),
        "blobD": np.ascontiguousarray(blobD),
        "biases": np.concatenate(
            [
                b_hru[:G].reshape(8, P).T,
                (b_hru[G:] + 1.0).reshape(8, P).T,
                b_rhc.reshape(8, P).T,
                np.full((P, 1), -1.0, np.float32),
            ],
            axis=1,
        ).astype(np.float32),
    }

    in_maps = []
    for c in range(NCORES):
        bs = slice(c * BL, (c + 1) * BL)
        m = dict(shared)
        xc = kp_np(xT[:, bs], 2)
        hc = kp_np(hT[:, bs].astype(bf), 8)
        m["blobA"] = np.ascontiguousarray(np.concatenate([xc, hc, blobA_w], axis=1))
        m["hT"] = np.ascontiguousarray(hT[:, bs])
        m["srcT2"] = np.ascontiguousarray(srcT2[:, bs, :])
        m["srcSumT"] = np.ascontiguousarray(srcSumT[:, bs])
        in_maps.append(m)
    return in_maps


def kernel(**inputs):
    global LAST_EXEC_NS
    if "nc" not in _CACHE:
        _CACHE["nc"] = _build()
    nc = _CACHE["nc"]

    in_maps = _host_prep(inputs)
    trace = os.environ.get("BASS_KERNEL_TRACE", "0") == "1"
    res = bass_utils.run_bass_kernel_spmd(
        nc, in_maps, core_ids=list(range(NCORES)), trace=trace
    )
    LAST_EXEC_NS = res.exec_time_ns

    gen = np.empty((B, G), np.float32)
    fac = np.empty((B, F), np.float32)
    for c in range(NCORES):
        bs = slice(c * BL, (c + 1) * BL)
        gen[bs] = res.results[c]["genT_out"].T
        fac[bs] = res.results[c]["facT_out"].T
    return gen, fac


# revision 8
# speedup vs baseline: 1.1948x; 1.1948x over previous
"""LFADS GenGRU cell + source attention + factor readout, 8-way data-parallel
over batch on Trainium2 (Bass/Tile).

Layout: every activation lives in SBUF as [features -> partitions,
batch -> free] ("T-layout"). All weights are host-pre-transposed to
[in_features, out_features] so each matmul is
    outT[o_tile, b] = W_T[:, o_tile].T @ inT[:, b]
with the contraction on partitions and no on-chip transposes anywhere.

Attention softmax (over S=64 source steps) runs in [e -> partitions,
(b, s) -> free] layout and is engine-balanced:
  - gen_alpha[e, b] is accumulated straight into the score PSUM via a
    rank-128 matmul against an identity whose rhs AP broadcasts over s
    (lhsT = gen_alpha in natural [b, e] layout), so no DVE broadcast-add.
  - exp runs on ScalarE directly from a 4-bank PSUM group; relu is replaced
    by the exact identity exp(relu(x)) = 1 + relu(exp(x) - 1): the +1
    offsets become a constant 64 in the softmax denominator and a
    host-precomputed sum_s src term in the numerator. relu(ex-1)
    alternates between ScalarE and VectorE to balance the two engines.
  - ctx and the denominator come from one interleaved [b, {ex2, ex2*src}, s]
    tile, reduced over s with a short bf16 add-tree + one f32-output reduce.

Matmul operands are bf16 (fp32 PSUM accumulation); fp32 is kept on the
gen/u/c elementwise path so the `gen` output only sees matmul rounding.
"""

import os
import sys

sys.path.insert(0, "/opt/trn_rl_repo")

import numpy as np
import ml_dtypes

import concourse.bacc as bacc
import concourse.mybir as mybir
import concourse.tile as tile
from concourse import bass_utils

B, S, IN, G, SRC, F = 1024, 64, 256, 1024, 256, 128
NCORES = 8
BL = B // NCORES  # 128, batch per core == partition count
P = 128
CB = 8  # batch columns per attention chunk (CB * S = 512 = one PSUM bank)
NCH = BL // CB  # 16 chunks per e-tile
QC = 4  # chunks per quarter = one 4-bank PSUM group

F32 = mybir.dt.float32
BF16 = mybir.dt.bfloat16
AF = mybir.ActivationFunctionType
OP = mybir.AluOpType
X = mybir.AxisListType.X

_CACHE: dict = {}
LAST_EXEC_NS = None


def _mm(nc, ps, lhsT, rhs, first, last):
    nc.tensor.matmul(ps, lhsT, rhs, start=first, stop=last)


def _build():
    nc = bacc.Bacc("TRN2", target_bir_lowering=False, debug=False)

    def din(name, shape, dt=BF16):
        return nc.dram_tensor(name, shape, dt, kind="ExternalInput").ap()

    # blobA1: xT | hTbf | WxruT  (cols: 256 | 1024 | 4096); WhruT separate
    NA = 2 * BL + 8 * BL + 2 * 2 * G
    NW = 8 * 2 * G
    # blobB: WxcT | WrhcT | WagT | WasT | ident  (2048 | 8192 | 2048 | 512 | 128)
    NB = 2 * G + 8 * G + 8 * SRC + 2 * SRC + P
    # blobD: WogT | WosT | WfacT  (8192 | 2048 | 1024)
    ND = 8 * G + 2 * G + 8 * F
    blobA_d = din("blobA", (P, NA))
    whru_d = din("whruB", (P, NW))
    blobB_d = din("blobB", (P, NB))
    blobD_d = din("blobD", (P, ND))
    hT_d = din("hT", (G, BL), F32)
    src_d = din("srcT2", (SRC, BL, S))
    bias_d = din("biases", (P, 25), F32)
    ssum_d = din("srcSumT", (SRC, BL), F32)
    gen_d = nc.dram_tensor("genT_out", (G, BL), F32, kind="ExternalOutput").ap()
    fac_d = nc.dram_tensor("facT_out", (F, BL), F32, kind="ExternalOutput").ap()

    kp = lambda ap: ap.rearrange("(k p) n -> p k n", p=P)  # [K*P, N] -> [P, K, N]

    with tile.TileContext(nc) as tc:
        with (
            tc.tile_pool(name="weights", bufs=1) as wp,
            tc.tile_pool(name="acts", bufs=1) as app,
            tc.tile_pool(name="tmp", bufs=4) as tp,
            tc.tile_pool(name="combo_p", bufs=2) as cbp,
            tc.tile_pool(name="psum", bufs=2, space="PSUM") as psp,
        ):
            # ---- persistent loads: few big blob DMAs (SP issue is ~2.2us each) ----
            blobA = wp.tile([P, NA], BF16, name="blobA")
            nc.sync.dma_start(blobA[:], blobA_d[:])
            bias_sb = wp.tile([P, 25], F32, name="bias_sb")
            nc.sync.dma_start(bias_sb[:], bias_d[:])
            whru_sb_t = wp.tile([P, NW], BF16, name="whru_sb_t")
            nc.sync.dma_start(whru_sb_t[:], whru_d[:])
            hT_sb = wp.tile([P, 8, BL], F32, name="hT_sb")
            nc.sync.dma_start(hT_sb[:], kp(hT_d))
            blobB = wp.tile([P, NB], BF16, name="blobB")
            nc.sync.dma_start(blobB[:], blobB_d[:])
            src_sb = wp.tile([P, 2, BL, S], BF16, name="src_sb")
            nc.sync.dma_start(src_sb[:], src_d.rearrange("(k p) b s -> p k b s", p=P))
            ssum_sb = wp.tile([P, 2, BL], F32, name="ssum_sb")
            nc.sync.dma_start(ssum_sb[:], kp(ssum_d))
            blobD = wp.tile([P, ND], BF16, name="blobD")
            nc.sync.dma_start(blobD[:], blobD_d[:])

            def seg(blob, off, shape):
                n = 1
                for d in shape:
                    n *= d
                ap = blob[:, off:off + n]
                if len(shape) > 1:
                    ap = ap.rearrange("p (a b) -> p a b", b=shape[-1]) if len(shape) == 2 else ap.rearrange(
                        "p (a b c) -> p a b c", b=shape[1], c=shape[2])
                return ap, off + n

            o = 0
            xT_sb, o = seg(blobA, o, (2, BL))
            h_bf, o = seg(blobA, o, (8, BL))
            wxru_sb, o = seg(blobA, o, (2, 2 * G))
            whru_sb, _ = seg(whru_sb_t, 0, (8, 2 * G))
            o = 0
            wxc_sb, o = seg(blobB, o, (2, G))
            wrhc_sb, o = seg(blobB, o, (8, G))
            wag_sb, o = seg(blobB, o, (8, SRC))
            was_sb, o = seg(blobB, o, (2, SRC))
            ident_sb = blobB[:, o:o + P]; o += P
            o = 0
            wog_sb, o = seg(blobD, o, (8, G))
            wos_sb, o = seg(blobD, o, (2, G))
            wfac_sb, o = seg(blobD, o, (8, F))

            rh_bf = app.tile([P, 8, BL], BF16, name="rh_bf")
            u_f = app.tile([P, 8, BL], F32, name="u_f")
            gen_f = app.tile([P, 8, BL], F32, name="gen_f")
            gen_bf = app.tile([P, 8, BL], BF16, name="gen_bf")

            # ---- ru = x @ WxruT + h @ WhruT + b; r = sig(ru_r), u = sig(ru_u + 1) ----
            for m in range(16):
                if m % 2 == 0:
                    rups = psp.tile([P, 16 * P], F32, tag="ps", name=f"rups{m}")
                sl = slice((m % 2) * 512, (m % 2) * 512 + P)
                wsl = slice(m * P, (m + 1) * P)
                for k in range(2):
                    _mm(nc, rups[:, sl], wxru_sb[:, k, wsl], xT_sb[:, k, :], k == 0, False)
                for k in range(8):
                    _mm(nc, rups[:, sl], whru_sb[:, k, wsl], h_bf[:, k, :], False, k == 7)
                if m < 8:
                    r_t = tp.tile([P, BL], F32, tag="r_t")
                    nc.scalar.activation(r_t[:], rups[:, sl], AF.Sigmoid, bias=bias_sb[:, m:m + 1])
                    nc.vector.tensor_tensor(rh_bf[:, m, :], r_t[:], hT_sb[:, m, :], op=OP.mult)
                else:
                    nc.scalar.activation(u_f[:, m - 8, :], rups[:, sl], AF.Sigmoid, bias=bias_sb[:, m:m + 1])

            # ---- c = tanh(x @ WxcT + rh @ WrhcT + b); gen = clip(c + u*(h-c)) ----
            for m in range(8):
                if m % 2 == 0:
                    cps = psp.tile([P, 16 * P], F32, tag="ps", name=f"cps{m}")
                sl = slice((m % 2) * 512, (m % 2) * 512 + P)
                wsl = slice(m * P, (m + 1) * P)
                for k in range(2):
                    _mm(nc, cps[:, sl], wxc_sb[:, k, wsl], xT_sb[:, k, :], k == 0, False)
                for k in range(8):
                    _mm(nc, cps[:, sl], wrhc_sb[:, k, wsl], rh_bf[:, k, :], False, k == 7)
                c_t = tp.tile([P, BL], F32, tag="c_t")
                nc.scalar.activation(c_t[:], cps[:, sl], AF.Tanh, bias=bias_sb[:, 16 + m:17 + m])
                d_t = tp.tile([P, BL], F32, tag="d_t")
                nc.vector.tensor_tensor(d_t[:], hT_sb[:, m, :], c_t[:], op=OP.subtract)
                nc.vector.tensor_tensor(d_t[:], u_f[:, m, :], d_t[:], op=OP.mult)
                nc.vector.tensor_tensor(gen_f[:, m, :], c_t[:], d_t[:], op=OP.add)
                nc.vector.tensor_scalar(gen_f[:, m, :], gen_f[:, m, :], 5.0, -5.0, op0=OP.min, op1=OP.max)
                nc.scalar.copy(gen_bf[:, m, :], gen_f[:, m, :])
            nc.sync.dma_start(kp(gen_d), gen_f[:])

            # ---- gen_alpha in natural [b, e] layout: lhsT = genT tiles ----
            gaps_t = psp.tile([P, 16 * P], F32, tag="ps", name="gaps")
            for k in range(8):
                _mm(nc, gaps_t[:, :SRC], gen_bf[:, k, :], wag_sb[:, k, :], k == 0, k == 7)
            ga_nat = app.tile([P, SRC], BF16, name="ga_nat")
            nc.scalar.copy(ga_nat[:], gaps_t[:, :SRC])

            # ---- attention: psum scores -> exp -> relu(ex-1) -> interleaved reduce ----
            # se_ct[et][:, b, 0] = sum_s relu(ex-1);  [..., 1] = sum_s relu(ex-1)*src
            se_ct = [app.tile([P, BL, 2], F32, name=f"se_ct{et}") for et in range(2)]
            NQ = NCH // QC
            for et in range(2):
                esl = slice(et * P, (et + 1) * P)
                for q in range(NQ):
                    chs = [q * QC + i for i in range(QC)]
                    qbs = slice(q * QC * CB, (q + 1) * QC * CB)  # 32 b columns
                    ps = psp.tile([P, QC * CB * S], F32, tag="ps", name=f"aps{et}_{q}")
                    for dk in range(2):
                        for i, ch in enumerate(chs):
                            rhs = src_sb[:, dk, ch * CB:(ch + 1) * CB, :].rearrange("p b s -> p (b s)")
                            _mm(nc, ps[:, i * CB * S:(i + 1) * CB * S], was_sb[:, dk, esl], rhs, dk == 0, False)
                    for i, ch in enumerate(chs):
                        rhs = ident_sb[:, ch * CB:(ch + 1) * CB].unsqueeze(2).broadcast_to([P, CB, S])
                        _mm(nc, ps[:, i * CB * S:(i + 1) * CB * S], ga_nat[:, esl], rhs, False, True)
                    ex = tp.tile([P, QC * CB, S], BF16, tag="ex")
                    nc.scalar.activation(ex[:], ps.rearrange("p (b s) -> p b s", s=S), AF.Exp)
                    combo = cbp.tile([P, QC * CB, 2, S], BF16, tag="combo")
                    if q % 2 == 0:
                        nc.vector.tensor_scalar(combo[:, :, 0, :], ex[:], 1.0, 0.0, op0=OP.subtract, op1=OP.max)
                    else:
                        nc.scalar.activation(combo[:, :, 0, :], ex[:], AF.Relu, bias=bias_sb[:, 24:25])
                    nc.vector.tensor_tensor(combo[:, :, 1, :], combo[:, :, 0, :], src_sb[:, et, qbs, :], op=OP.mult)
                    # reduce over s: short bf16 add-tree, then one f32-output reduce
                    nc.vector.tensor_tensor(combo[:, :, :, 0:32], combo[:, :, :, 0:32], combo[:, :, :, 32:64], op=OP.add)
                    nc.vector.tensor_tensor(combo[:, :, :, 0:16], combo[:, :, :, 0:16], combo[:, :, :, 16:32], op=OP.add)
                    nc.vector.tensor_tensor(combo[:, :, :, 0:8], combo[:, :, :, 0:8], combo[:, :, :, 8:16], op=OP.add)
                    nc.vector.reduce_sum(se_ct[et][:, qbs, :], combo[:, :, :, 0:8], axis=X)

            # softmax epilogue:
            #   sumexp = 64 + sum relu(ex-1);  ctx = (src_sum + sum relu(ex-1)*src) / sumexp
            ctx_bf = app.tile([P, 2, BL], BF16, name="ctx_bf")
            for et in range(2):
                den = tp.tile([P, BL], F32, tag="den")
                nc.vector.tensor_scalar(den[:], se_ct[et][:, :, 0], float(S), None, op0=OP.add)
                rcp = tp.tile([P, BL], F32, tag="rcp")
                nc.vector.reciprocal(rcp[:], den[:])
                num = tp.tile([P, BL], F32, tag="num")
                nc.vector.tensor_tensor(num[:], ssum_sb[:, et, :], se_ct[et][:, :, 1], op=OP.add)
                nc.vector.tensor_tensor(ctx_bf[:, et, :], num[:], rcp[:], op=OP.mult)

            # ---- attn_out = tanh(WogT.T @ genT + WosT.T @ ctxT) ----
            attn_bf = app.tile([P, 8, BL], BF16, name="attn_bf")
            for m in range(8):
                if m % 2 == 0:
                    aops = psp.tile([P, 16 * P], F32, tag="ps", name=f"aops{m}")
                sl = slice((m % 2) * 512, (m % 2) * 512 + P)
                wsl = slice(m * P, (m + 1) * P)
                for k in range(8):
                    _mm(nc, aops[:, sl], wog_sb[:, k, wsl], gen_bf[:, k, :], k == 0, False)
                for k in range(2):
                    _mm(nc, aops[:, sl], wos_sb[:, k, wsl], ctx_bf[:, k, :], False, k == 1)
                nc.scalar.activation(attn_bf[:, m, :], aops[:, sl], AF.Tanh)

            # ---- factors = WfacT.T @ attn_outT ----
            fps = psp.tile([P, 16 * P], F32, tag="ps", name="fps")
            for k in range(8):
                _mm(nc, fps[:, :P], wfac_sb[:, k, :], attn_bf[:, k, :], k == 0, k == 7)
            fac_sb = app.tile([P, BL], F32, name="fac_sb")
            nc.scalar.copy(fac_sb[:], fps[:, :P])
            nc.sync.dma_start(fac_d[:], fac_sb[:])

    nc.compile()
    return nc


def _host_prep(inputs):
    bf = ml_dtypes.bfloat16

    def t(a):
        return np.ascontiguousarray(np.asarray(a, dtype=np.float32).T)

    x = np.asarray(inputs["x"], np.float32)
    h = np.asarray(inputs["h"], np.float32)
    src = np.asarray(inputs["src"], np.float32)
    W_alpha = np.asarray(inputs["W_alpha"], np.float32)
    W_out = np.asarray(inputs["W_out"], np.float32)
    b_hru = np.asarray(inputs["b_hru"], np.float32)
    b_rhc = np.asarray(inputs["b_rhc"], np.float32)

    xT = t(x).astype(bf)                       # [IN, B]
    hT = t(h)                                  # [G, B] fp32
    srcT2 = np.ascontiguousarray(src.transpose(2, 1, 0)).astype(bf)  # [SRC, B, S]
    # the kernel multiplies attention weights against bf16 src, and the
    # numerator offset must match that rounding exactly
    srcSumT = srcT2.astype(np.float32).sum(axis=2)  # [SRC, B] fp32

    def kp_np(wt, K):
        # [K*P, N] -> [P, K*N]
        a = np.asarray(wt)
        N = a.shape[1]
        return a.reshape(K, P, N).transpose(1, 0, 2).reshape(P, K * N)

    WxruT = t(inputs["W_xru"]).astype(bf)
    WhruT = t(inputs["W_hru"]).astype(bf)
    WxcT = t(inputs["W_xc"]).astype(bf)
    WrhcT = t(inputs["W_rhc"]).astype(bf)
    WagT = np.ascontiguousarray(W_alpha[:, :G].T).astype(bf)
    WasT = np.ascontiguousarray(W_alpha[:, G:].T).astype(bf)
    WogT = np.ascontiguousarray(W_out[:, :G].T).astype(bf)
    WosT = np.ascontiguousarray(W_out[:, G:].T).astype(bf)
    WfacT = t(inputs["W_fac"]).astype(bf)
    identb = np.eye(P, dtype=np.float32).astype(bf)

    blobB = np.concatenate(
        [kp_np(WxcT, 2), kp_np(WrhcT, 8), kp_np(WagT, 8), kp_np(WasT, 2), identb],
        axis=1)
    blobD = np.concatenate(
        [kp_np(WogT, 8), kp_np(WosT, 2), kp_np(WfacT, 8)], axis=1)
    blobA_w = kp_np(WxruT, 2)

    shared = {
        "blobB": np.ascontiguousarray(blobB),
        "whruB": np.ascontiguousarray(kp_np(WhruT, 8)),
        "blobD": np.ascontiguousarray(blobD),
        "biases": np.concatenate(
            [
                b_hru[:G].reshape(8, P).T,
                (b_hru[G:] + 1.0).reshape(8, P).T,
                b_rhc.reshape(8, P).T,
                np.full((P, 1), -1.0, np.float32),
            ],
            axis=1,
        ).astype(np.float32),
    }

    in_maps = []
    for c in range(NCORES):
        bs = slice(c * BL, (c + 1) * BL)
        m = dict(shared)
        xc = kp_np(xT[:, bs], 2)
        hc = kp_np(hT[:, bs].astype(bf), 8)
        m["blobA"] = np.ascontiguousarray(np.concatenate([xc, hc, blobA_w], axis=1))
        m["hT"] = np.ascontiguousarray(hT[:, bs])
        m["srcT2"] = np.ascontiguousarray(srcT2[:, bs, :])
        m["srcSumT"] = np.ascontiguousarray(srcSumT[:, bs])
        in_maps.append(m)
    return in_maps


def kernel(**inputs):
    global LAST_EXEC_NS
    if "nc" not in _CACHE:
        _CACHE["nc"] = _build()
    nc = _CACHE["nc"]

    in_maps = _host_prep(inputs)
    trace = os.environ.get("BASS_KERNEL_TRACE", "0") == "1"
    res = bass_utils.run_bass_kernel_spmd(
        nc, in_maps, core_ids=list(range(NCORES)), trace=trace
    )
    LAST_EXEC_NS = res.exec_time_ns

    gen = np.empty((B, G), np.float32)
    fac = np.empty((B, F), np.float32)
    for c in range(NCORES):
        bs = slice(c * BL, (c + 1) * BL)
        gen[bs] = res.results[c]["genT_out"].T
        fac[bs] = res.results[c]["facT_out"].T
    return gen, fac
